# revision 4
# baseline (speedup 1.0000x reference)
"""Diagonal reservoir RNN (DRNN) Trainium2 kernel.

Computes: U = einsum('ri,ti->tr', W_in, x[:,:,0]);  s_t = tanh(u_t + d * s_{t-1})
Returns states [T, RES, 1].

Strategy (v2)
-------------
Shard the reservoir dim (RES=4096) across 8 cores (512 units each, as 4
groups of 128 partitions).  Layout on device: units on partitions, time on
the free axis.

The sequential scan is evaluated by Picard (fixed-point) iteration:

    y^0 = tanh(d * V)                       (warm start)
    y^{k+1}_t = tanh(d * (y^k_{t-1} + V_t))  where V = U / d (folded into W)

v2 changes over the 446us baseline:
  * Reservoir units are sorted by |d| on the host and assigned to partition
    groups by quartile.  Picard contraction is ~|d * tanh'| per step, so
    low-|d| groups converge in fewer sweeps: per-group sweep counts
    (4, 5, 5, 6) instead of a uniform 7 (measured err 7.9e-3 vs 2e-2 gate).
  * Scan state (y, V, w) is bf16.  The shift-by-one add y_{t-1} + V_t is
    layed out with y-data on even columns and V/w-data on odd columns so
    all three APs of the big DVE add are 4-byte aligned -> 2x bf16 mode.
  * Warm-start tanh reads the GEMM result directly from PSUM (per 512-wide
    sub-tile) so it runs during the chunk's GEMM instead of after it.
  * PSUM->SBUF copies all on DVE (casting to bf16); ACT does only tanh.
  * Output is stored/DMAed as bf16 (scale ~1.0, gate 2e-2) halving out DMA.
  * One fewer total sweep (warm+5 max) than baseline's warm+6.

The GEMM is the proven 3-term bf16 split (W ~ Wh+Wl, x ~ xh+xl, dropping
lo*lo): cheaper splits (2-term, fp8-corrected) measure 9e-2..2e-1 error
after recurrence amplification - over the gate.  PE time ~328us/core is
the compute floor; the scan pipelines behind it in time chunks with exact
carry of each group's final state column between chunks.
"""

import ml_dtypes
import numpy as np

import concourse.bass as bass
import concourse.mybir as mybir
import concourse.tile as tile
from concourse import bacc
from concourse.bass_utils import run_bass_kernel_spmd

T = 8192
INPUT = 1024
RES = 4096
NCORES = 8
RS = RES // NCORES          # 512 units per core
G = RS // 128               # 4 partition groups per core (|d| quartiles)
KT = INPUT // 128           # 8 contraction tiles
CHUNKS = (2048, 2048, 1536, 1024, 1024, 512)
SUB = 512                   # matmul moving-operand width (one PSUM bank fp32)
SWEEPS = (4, 5, 5, 6)       # total tanh sweeps (incl. warm) per |d| quartile

F32 = mybir.dt.float32
BF16 = mybir.dt.bfloat16


def _emit(nc: bass.Bass, tc: tile.TileContext, x_hi, x_lo, w_hl, d_c, s_t):
    Tanh = mybir.ActivationFunctionType.Tanh
    assert sum(CHUNKS) == T
    SMAX = max(SWEEPS)
    with (
        tc.tile_pool(name="const", bufs=1) as constp,
        tc.tile_pool(name="xin", bufs=32) as xp,
        tc.tile_pool(name="vbuf", bufs=2) as vp,
        tc.tile_pool(name="ybuf", bufs=3) as yp,
        tc.tile_pool(name="wbuf", bufs=2) as wp,
        tc.tile_pool(name="carry", bufs=2) as cp,
        tc.tile_pool(name="psum", bufs=8, space="PSUM") as pp,
    ):
        # Weights: w_hl is [128, 2*KT*RS] bf16, host-packed so that
        #   hi tile (g,k) = w_sb[:, k*RS + g*128 +: 128]
        #   lo tile (g,k) = w_sb[:, KT*RS + k*RS + g*128 +: 128]
        # Load on the (otherwise idle) GPSIMD queue, k-sliced hi-first, so
        # the first matmuls start as soon as whi(k=0) + x(k=0) land instead
        # of waiting out the whole 4.2MB weight block behind x on one queue.
        w_sb = constp.tile([128, 2 * KT * RS], BF16)
        d_sb = constp.tile([128, G], F32)
        nc.gpsimd.dma_start(d_sb[:], d_c[:])
        for k in range(2 * KT):
            nc.gpsimd.dma_start(w_sb[:, k * RS : (k + 1) * RS],
                                w_hl[:, k * RS : (k + 1) * RS])

        # Preload the ACT tanh table set while initial DMAs run.
        dummy = constp.tile([128, 1], F32)
        nc.vector.memset(dummy[:], 0.0)
        nc.scalar.activation(dummy[:], dummy[:], Tanh)

        carries = []
        for g in range(G):
            cg = cp.tile([128, 1], BF16, tag=f"c{g}")
            nc.vector.memset(cg[:], 0.0)
            carries.append(cg)

        t0 = 0
        for c, TC in enumerate(CHUNKS):
            nsub = TC // SUB
            # V' tiles: data column t at tile column t+1 (odd start) so the
            # iteration adds are 4B-aligned; y tiles: data column t at t.
            vg = [vp.tile([128, TC + 1], BF16, tag=f"v{g}", name=f"v{g}")
                  for g in range(G)]
            y0 = [yp.tile([128, TC], BF16, tag=f"y{g}", name=f"y{g}w")
                  for g in range(G)]

            # ---- GEMM: V[g] = Wh@xh + Wh@xl + Wl@xh, K accumulated in PSUM
            for sub in range(nsub):
                xts = []
                for k in range(KT):
                    xh = xp.tile([128, SUB], BF16, tag="x", name="xh")
                    nc.sync.dma_start(
                        xh[:],
                        x_hi[k * 128 : (k + 1) * 128,
                             t0 + sub * SUB : t0 + (sub + 1) * SUB],
                    )
                    xl = xp.tile([128, SUB], BF16, tag="x", name="xl")
                    nc.sync.dma_start(
                        xl[:],
                        x_lo[k * 128 : (k + 1) * 128,
                             t0 + sub * SUB : t0 + (sub + 1) * SUB],
                    )
                    xts.append((xh, xl))
                for g in range(G):
                    ps = pp.tile([128, SUB], F32, tag="ps", name="ps")
                    # hi-weight terms first (only need the hi DMA block)
                    for k in range(KT):
                        whi = w_sb[:, k * RS + g * 128 : k * RS + (g + 1) * 128]
                        xh, xl = xts[k]
                        nc.tensor.matmul(ps[:], whi, xh[:],
                                         start=(k == 0), stop=False)
                        nc.tensor.matmul(ps[:], whi, xl[:],
                                         start=False, stop=False)
                    for k in range(KT):
                        wlo = w_sb[:, KT * RS + k * RS + g * 128
                                   : KT * RS + k * RS + (g + 1) * 128]
                        xh, _ = xts[k]
                        nc.tensor.matmul(ps[:], wlo, xh[:],
                                         start=False, stop=(k == KT - 1))
                    # V' copy (DVE, cast to bf16, odd-column destination)
                    nc.vector.tensor_copy(
                        vg[g][:, 1 + sub * SUB : 1 + (sub + 1) * SUB], ps[:])
                    # warm-start tanh straight from PSUM (ACT)
                    nc.scalar.activation(
                        y0[g][:, sub * SUB : (sub + 1) * SUB], ps[:], Tanh,
                        scale=d_sb[:, g : g + 1])

            # ---- Picard iterations (per-group sweep counts)
            ycur = list(y0)
            for j in range(1, SMAX):
                for g in range(G):
                    if SWEEPS[g] <= j:
                        continue
                    w = wp.tile([128, TC + 1], BF16, tag=f"w{g}", name=f"w{g}")
                    nc.vector.tensor_add(w[:, 1:2], carries[g][:],
                                         vg[g][:, 1:2])
                    nc.vector.tensor_add(w[:, 2 : TC + 1],
                                         ycur[g][:, 0 : TC - 1],
                                         vg[g][:, 2 : TC + 1])
                    ynew = yp.tile([128, TC], BF16, tag=f"y{g}", name=f"y{g}")
                    nc.scalar.activation(ynew[:], w[:, 1 : TC + 1], Tanh,
                                         scale=d_sb[:, g : g + 1])
                    ycur[g] = ynew

            # ---- carries + output
            for g in range(G):
                cg = cp.tile([128, 1], BF16, tag=f"c{g}")
                nc.vector.tensor_copy(cg[:], ycur[g][:, TC - 1 : TC])
                carries[g] = cg
                nc.gpsimd.dma_start(
                    s_t[g * 128 : (g + 1) * 128, t0 : t0 + TC], ycur[g][:])
            t0 += TC


_NC_CACHE = None


def _build_nc() -> bass.Bass:
    global _NC_CACHE
    if _NC_CACHE is None:
        nc = bacc.Bacc(trn_type="TRN2")
        x_hi = nc.dram_tensor("x_hi", [INPUT, T], BF16, kind="ExternalInput")
        x_lo = nc.dram_tensor("x_lo", [INPUT, T], BF16, kind="ExternalInput")
        w_hl = nc.dram_tensor("w_hl", [128, 2 * KT * RS], BF16,
                              kind="ExternalInput")
        d_c = nc.dram_tensor("d_c", [128, G], F32, kind="ExternalInput")
        s_t = nc.dram_tensor("s_t", [RS, T], BF16, kind="ExternalOutput")
        with tile.TileContext(nc) as tc:
            _emit(nc, tc, x_hi, x_lo, w_hl, d_c, s_t)
        nc.compile()
        _NC_CACHE = nc
    return _NC_CACHE


def _pack_w(wc):
    """wc: [RS, INPUT] fp32 -> [128, KT*RS] in SBUF layout (p, then k, m)."""
    return np.ascontiguousarray(
        wc.T.reshape(KT, 128, RS).transpose(1, 0, 2).reshape(128, KT * RS))


def _make_in_maps(x, W_in, d):
    bf16 = ml_dtypes.bfloat16
    x = np.asarray(x, dtype=np.float32)
    W_in = np.asarray(W_in, dtype=np.float32)
    d = np.asarray(d, dtype=np.float32)
    x2 = x.reshape(T, INPUT)
    x_t = np.ascontiguousarray(x2.T)                       # [INPUT, T]
    x_hi = x_t.astype(bf16)
    x_lo = (x_t - x_hi.astype(np.float32)).astype(bf16)

    # Sort reservoir units by |d|; quartile q -> partition group q on every
    # core.  Core i, group g, partition p <-> sorted unit g*1024 + i*128 + p.
    perm = np.argsort(np.abs(d), kind="stable")            # sorted -> original
    W_s = W_in[perm]
    d_s = d[perm]
    wp_all = (W_s / d_s[:, None]).astype(np.float32)       # fold 1/d into W

    in_maps = []
    for i in range(NCORES):
        rows = np.concatenate(
            [np.arange(g * 1024 + i * 128, g * 1024 + (i + 1) * 128)
             for g in range(G)])
        wc = wp_all[rows]                                  # [RS, INPUT]
        wc_hi = wc.astype(bf16).astype(np.float32)
        wc_lo = wc - wc_hi
        w_hl = np.concatenate(
            [_pack_w(wc_hi), _pack_w(wc_lo)], axis=1).astype(bf16)
        w_hl = np.ascontiguousarray(w_hl)
        d_cols = np.ascontiguousarray(
            d_s[rows].reshape(G, 128).T)                   # [128, G]
        in_maps.append({"x_hi": x_hi, "x_lo": x_lo, "w_hl": w_hl,
                        "d_c": d_cols})
    return in_maps, perm


def _run(x, W_in, d, **spmd_kwargs):
    nc = _build_nc()
    in_maps, perm = _make_in_maps(x, W_in, d)
    res = run_bass_kernel_spmd(nc, in_maps, core_ids=list(range(NCORES)),
                               **spmd_kwargs)
    # shards[i] is [RS, T] bf16 holding sorted units g*1024 + i*128 + p
    sorted_full = np.empty((RES, T), dtype=np.float32)
    for i in range(NCORES):
        sh = np.asarray(res.results[i]["s_t"]).astype(np.float32)
        for g in range(G):
            sorted_full[g * 1024 + i * 128 : g * 1024 + (i + 1) * 128] = (
                sh[g * 128 : (g + 1) * 128])
    full = np.empty((RES, T), dtype=np.float32)
    full[perm] = sorted_full                               # unpermute units
    out = np.ascontiguousarray(full.T)[:, :, None].astype(np.float32)
    return out, res


def kernel(x, W_in, d):
    out, _ = _run(x, W_in, d)
    return out
